# revision 19
# baseline (speedup 1.0000x reference)
"""Transformer block (dense_transformer) on 8 TRN2 NeuronCores.

Strategy: pure data-parallel over batch (B=128 -> 16 items/core), weights
replicated. Per item, all linear layers run feature-major ([feat, T] with
feat on partitions); LayerNorm/softmax run token-major ([T, feat]).
Matmul datapath is bf16 (2x PE stream rate vs f32r); residual stream and
softmax statistics stay f32. The softmax normalize is fused into the
weight transpose on PE via a diagonal 1/rowsum matrix built on GpSimd.
LN stats for group g+1 are computed during group g so PE never idles at
group boundaries.
"""

import numpy as np
import ml_dtypes

import concourse.bass as bass
import concourse.mybir as mybir
from concourse.tile import TileContext
from concourse.vector_clock import ScopedClock

F32 = mybir.dt.float32
BF16 = mybir.dt.bfloat16
AF = mybir.ActivationFunctionType
AX = mybir.AxisListType
ALU = mybir.AluOpType

B, T, C, H, D = 128, 256, 384, 6, 64
F = 4 * C
NCORES = 8
BL = B // NCORES
P = 128
TT = T // P    # 2 token tiles
CT = C // P    # 3 channel tiles
FT = F // P    # 12 ffn-hidden tiles
H2 = H // 2    # head pairs
LN_EPS = 1e-5
CSCALE = float(C) ** -0.5
NEG = -1.0e9


class PatchedTileContext(TileContext):
    """Workaround for this container's walrus: BIR instructions may carry at
    most ONE attached sem wait. Hoist extras into standalone waits."""

    def _hoist_multi_waits(self):
        nc = self.nc
        assert self.sems is not None
        sem_by_num = {s.num: s for s in self.sems.allocated().values()}
        for func in nc.m.functions:
            for blk in func.blocks:
                insts = blk.instructions
                i = 0
                while i < len(insts):
                    inst = insts[i]
                    si = inst.sync_info
                    waits = list(si.on_wait) if (si and si.on_wait) else []
                    if len(waits) <= 1:
                        i += 1
                        continue
                    hoist = waits[1:]
                    for w in hoist:
                        if not (
                            w.sync_type == "semaphore"
                            and w.wait_mode == "sem-ge-imm"
                            and w.id in sem_by_num
                        ):
                            raise RuntimeError(
                                f"cannot hoist waits on {inst.name}: {waits}"
                            )
                    del si.on_wait[1:]
                    engine = nc.engines[inst.engine]
                    new_insts = []
                    for w in hoist:
                        wi = engine.wait_ge(sem_by_num[w.id], w.wait_value)
                        new_insts.append(wi.ins)
                    cur_list = nc.cur_bb.bb.instructions
                    for ni in new_insts:
                        cur_list.remove(ni)
                    insts[i:i] = new_insts
                    i += len(new_insts) + 1

    def _drain_and_barrier(self, tick_clock, wait_clock):
        nc = self.nc
        self._hoist_multi_waits()

        drain_inst = nc.sync.drain()
        wait_clock.add_sem_waits(
            drain_inst.ins, ScopedClock({None: tick_clock.global_clock})
        )
        waits = list(drain_inst.ins.sync_info.on_wait or [])
        if len(waits) > 1:
            drain_inst.ins.sync_info.on_wait.clear()
            assert self.sems is not None
            sem_by_num = {s.num: s for s in self.sems.allocated().values()}
            new_waits = []
            for w in waits:
                assert w.sync_type == "semaphore" and w.wait_mode == "sem-ge-imm", w
                new_waits.append(nc.sync.wait_ge(sem_by_num[w.id], w.wait_value))
            bb = nc.cur_bb.bb
            insts = bb.instructions
            names = [i.name for i in insts]
            di = names.index(drain_inst.ins.name)
            tail = insts[di + 1 : di + 1 + len(new_waits)]
            assert len(tail) == len(new_waits)
            insts[di : di + 1 + len(new_waits)] = tail + [drain_inst.ins]

        nc.all_engine_barrier()
        assert self.sems is not None
        popped = nc._tile_sem_poison_stack.pop()
        assert popped is self._sem_poison
        nc.clear_and_free_semaphores(list(self.sems.allocated().values()))
        nc.all_engine_barrier()


def ts(i, n=P):
    return slice(i * n, (i + 1) * n)


def build_nc():
    nc = bass.Bass()
    x_in = nc.dram_tensor("x", [BL, T, C], F32, kind="ExternalInput")
    wq_in = nc.dram_tensor("wqf", [C, C], BF16, kind="ExternalInput")
    wk_in = nc.dram_tensor("wkf", [C, C], BF16, kind="ExternalInput")
    wv_in = nc.dram_tensor("wvf", [C, C], BF16, kind="ExternalInput")
    wp_in = nc.dram_tensor("wpf", [C, C], BF16, kind="ExternalInput")
    w1_in = nc.dram_tensor("w1f", [C, F], BF16, kind="ExternalInput")
    w2_in = nc.dram_tensor("w2f", [F, C], BF16, kind="ExternalInput")
    gb_in = nc.dram_tensor("gb", [6, C], F32, kind="ExternalInput")
    b1_in = nc.dram_tensor("b1v", [F], F32, kind="ExternalInput")
    id_in = nc.dram_tensor("ident", [P, P], BF16, kind="ExternalInput")
    tri_in = nc.dram_tensor("tri3", [P, 3 * P], BF16, kind="ExternalInput")
    out_t = nc.dram_tensor("out", [BL, T, C], F32, kind="ExternalOutput")

    IP = 2               # items per group
    NG = BL // IP        # groups
    W = IP * T           # moving width for feature-major matmuls (512)

    with PatchedTileContext(nc) as tc:
        with tc.tile_pool(name="consts", bufs=1) as consts:
            def load_w(ap_dram, kt, m, tag):
                w = consts.tile([P, kt, m], BF16, tag=tag)
                nc.sync.dma_start(w[:], ap_dram.rearrange("(kt p) m -> p kt m", p=P))
                return w

            wq_r = load_w(wq_in[:], CT, C, "wq")
            wk_r = load_w(wk_in[:], CT, C, "wk")
            wv_r = load_w(wv_in[:], CT, C, "wv")
            wp_r = load_w(wp_in[:], CT, C, "wp")
            w1_r = load_w(w1_in[:], CT, F, "w1")
            w2_r = load_w(w2_in[:], FT, C, "w2")

            id_b = consts.tile([P, P], BF16, tag="idb")
            nc.sync.dma_start(id_b[:], id_in[:])
            tri3 = consts.tile([P, 3, P], BF16, tag="tri3")
            nc.sync.dma_start(tri3[:], tri_in.rearrange("p (b f) -> p b f", b=3))
            ones_t = consts.tile([P, P], BF16, tag="ones")
            nc.gpsimd.memset(ones_t[:], 1.0)
            gb = consts.tile([P, 6, CT], F32, tag="gb")
            nc.sync.dma_start(gb[:], gb_in.rearrange("g (ct p) -> p g ct", p=P))
            b1c = consts.tile([P, FT], F32, tag="b1c")
            nc.sync.dma_start(b1c[:], b1_in.rearrange("(ft p) -> p ft", p=P))
            epsc = consts.tile([P, 1], F32, tag="eps")
            nc.gpsimd.memset(epsc[:], LN_EPS)

            g1c = gb[:, 0, :]
            be1c = gb[:, 1, :]
            g2c = gb[:, 2, :]
            be2c = gb[:, 3, :]
            bpc = gb[:, 4, :]
            b2c = gb[:, 5, :]

            with (
                tc.tile_pool(name="xld", bufs=3) as xldp,
                tc.tile_pool(name="act", bufs=2) as actp,
                tc.tile_pool(name="xn", bufs=3) as xnp,
                tc.tile_pool(name="fm", bufs=2) as fmp,
                tc.tile_pool(name="zp", bufs=1) as zp,
                tc.tile_pool(name="attn", bufs=4) as attnp,
                tc.tile_pool(name="stats", bufs=8) as stats,
                tc.tile_pool(name="ps5", bufs=2, space="PSUM") as ps5,
                tc.tile_pool(name="ps2", bufs=2, space="PSUM") as ps2,
            ):
                def load_x(g):
                    xt = xldp.tile([P, IP, TT, C], F32, tag="x")
                    nc.sync.dma_start(
                        xt[:],
                        x_in[g * IP : (g + 1) * IP].rearrange(
                            "i (tt p) c -> p i tt c", p=P
                        ),
                    )
                    return xt

                def ln_stats_gen(src):
                    """src [P, IP, TT, C] f32 -> xn [P, 4, C] bf16 normalized
                    (no affine -- g/be folded into the ln_fm copies)."""
                    s4 = src.rearrange("p i tt c -> p (i tt) c")
                    nseg = IP * TT
                    bns = stats.tile([P, nseg, 6], F32, tag="bns")
                    for seg in range(nseg):
                        nc.vector.bn_stats(bns[:, seg, :], s4[:, seg, :])
                        if seg % 2 == 1:
                            yield
                    mv = stats.tile([P, nseg, 2], F32, tag="mv")
                    for seg in range(nseg):
                        nc.vector.bn_aggr(mv[:, seg, :], bns[:, seg, :])
                    yield
                    lnv = stats.tile([P, nseg], F32, tag="lnv")
                    nc.scalar.activation(lnv[:], mv[:, :, 1], AF.Ln, bias=epsc[:])
                    rstd = stats.tile([P, nseg], F32, tag="rstd")
                    nc.scalar.activation(rstd[:], lnv[:], AF.Exp, scale=-0.5)
                    yield
                    xn = xnp.tile([P, nseg, C], BF16, tag="xn")
                    for seg in range(nseg):
                        nc.vector.tensor_scalar(
                            xn[:, seg, :], s4[:, seg, :],
                            mv[:, seg, 0:1],
                            rstd[:, seg : seg + 1],
                            ALU.subtract, ALU.mult,
                        )
                        if seg % 2 == 1:
                            yield
                    return xn

                def ln_fm(xn, gcol, becol, tag="hct"):
                    """xn [P, 4, C] bf16 -> h_ct [P, CT, IP, T] bf16 w/ affine."""
                    h_ct = fmp.tile([P, CT, IP, T], BF16, tag=tag, name="h_ct")
                    for ct in range(CT):
                        ps = ps5.tile([P, IP, T], BF16, tag="ps5t", bufs=2)
                        for i in range(IP):
                            for tt in range(TT):
                                nc.tensor.transpose(
                                    ps[:, i, ts(tt)],
                                    xn[:, i * TT + tt, ts(ct)],
                                    id_b[:],
                                )
                        nc.scalar.activation(
                            h_ct[:, ct, :, :], ps[:], AF.Identity,
                            bias=becol[:, ct : ct + 1],
                            scale=gcol[:, ct : ct + 1],
                        )
                        yield
                    return h_ct

                def front(g, x_t, h_ct):
                    qT = fmp.tile([P, CT, IP, T], BF16, tag="fmA")
                    kT = fmp.tile([P, CT, IP, T], BF16, tag="fmB")
                    for m in range(CT):
                        psq = ps5.tile([P, IP, T], F32, tag="ps5")
                        psk = ps5.tile([P, IP, T], F32, tag="ps5")
                        for k in range(CT):
                            nc.tensor.matmul(
                                psq[:], wq_r[:, k, ts(m)], h_ct[:, k, :, :],
                                start=(k == 0), stop=(k == CT - 1),
                            )
                            nc.tensor.matmul(
                                psk[:], wk_r[:, k, ts(m)], h_ct[:, k, :, :],
                                start=(k == 0), stop=(k == CT - 1),
                            )
                        nc.vector.tensor_copy(qT[:, m, :, :], psq[:])
                        nc.vector.tensor_copy(kT[:, m, :, :], psk[:])
                        yield
                    v_sb = fmp.tile([P, IP, TT, C], BF16, tag="fmC")
                    for i in range(IP):
                        for st in range(TT):
                            psv = ps2.tile([P, C], F32, tag="pss")
                            for k in range(CT):
                                nc.tensor.matmul(
                                    psv[:], h_ct[:, k, i, ts(st)], wv_r[:, k, :],
                                    start=(k == 0), stop=(k == CT - 1),
                                )
                            if st == 0:
                                nc.scalar.copy(v_sb[:, i, st, :], psv[:])
                            else:
                                nc.vector.tensor_copy(v_sb[:, i, st, :], psv[:])
                            yield

                    attnT = fmp.tile([P, CT, IP, T], BF16, tag="fmC2")
                    for i in range(IP):
                        for j in range(H2):
                            # heads A=2j (q/k/v at partitions/cols 0:64 of
                            # chunk j), B=2j+1 (64:128). Scores computed
                            # directly in [s, t] layout (stationary = kT
                            # s-block); head pair runs concurrently via PE
                            # row tiling (K=64 at row 0 / row 64).
                            hA, hB = 2 * j, 2 * j + 1
                            cA = slice(hA * 64, hA * 64 + 64)
                            cB = slice(hB * 64, hB * 64 + 64)
                            # pss layout [s, 3, 128]:
                            #  [:,0:2,:] = s-block0 x t 0:256
                            #  [:,2,:]   = s-block1 x t 128:256
                            pssA = ps2.tile([P, 3, P], F32, tag="pss")
                            pssB = ps2.tile([P, 3, P], F32, tag="pss")
                            nc.tensor.matmul(
                                pssA[:, 0:2, :], kT[0:64, j, i, ts(0)],
                                qT[0:64, j, i, :], start=True, stop=True,
                            )
                            nc.tensor.matmul(
                                pssB[:, 0:2, :], kT[64:128, j, i, ts(0)],
                                qT[64:128, j, i, :], start=True, stop=True,
                            )
                            nc.tensor.matmul(
                                pssA[:, 2, :], kT[0:64, j, i, ts(1)],
                                qT[0:64, j, i, ts(1)], start=True, stop=True,
                            )
                            nc.tensor.matmul(
                                pssB[:, 2, :], kT[64:128, j, i, ts(1)],
                                qT[64:128, j, i, ts(1)], start=True, stop=True,
                            )
                            weA = attnp.tile([P, 3, P], BF16, tag="weA")
                            weB = attnp.tile([P, 3, P], BF16, tag="weB")
                            nc.scalar.activation(
                                weA[:], pssA[:], AF.Exp, scale=CSCALE)
                            nc.scalar.activation(
                                weB[:], pssB[:], AF.Exp, scale=CSCALE)
                            # causal mask: zero the two diagonal blocks
                            # (tri3 = [tri | ones | tri])
                            nc.vector.tensor_tensor(
                                weA[:], weA[:], tri3[:], ALU.mult)
                            nc.vector.tensor_tensor(
                                weB[:], weB[:], tri3[:], ALU.mult)
                            yield
                            # Z[t] = sum_s w[s,t], broadcast to the head's 64
                            # partitions by a ones-matrix stationary (A rows
                            # 0:64, B rows 64:128 -> col-tiled concurrent)
                            psZb = ps2.tile([P, T], F32, tag="psa")
                            psaP = ps2.tile([P, T], F32, tag="psa")
                            nc.tensor.matmul(
                                psZb[0:64, :], ones_t[:, 0:64],
                                weA[:, 0:2, :], start=True, stop=False,
                            )
                            nc.tensor.matmul(
                                psZb[64:128, :], ones_t[:, 0:64],
                                weB[:, 0:2, :], start=True, stop=False,
                            )
                            nc.tensor.matmul(
                                psZb[0:64, P:], ones_t[:, 0:64],
                                weA[:, 2, :], start=False, stop=True,
                            )
                            nc.tensor.matmul(
                                psZb[64:128, P:], ones_t[:, 0:64],
                                weB[:, 2, :], start=False, stop=True,
                            )
                            # attn (unnormalized): out[d, t] = sum_s v w
                            nc.tensor.matmul(
                                psaP[0:64, :], v_sb[:, i, 0, cA],
                                weA[:, 0:2, :], start=True, stop=False,
                            )
                            nc.tensor.matmul(
                                psaP[64:128, :], v_sb[:, i, 0, cB],
                                weB[:, 0:2, :], start=True, stop=False,
                            )
                            nc.tensor.matmul(
                                psaP[0:64, P:], v_sb[:, i, 1, cA],
                                weA[:, 2, :], start=False, stop=True,
                            )
                            nc.tensor.matmul(
                                psaP[64:128, P:], v_sb[:, i, 1, cB],
                                weB[:, 2, :], start=False, stop=True,
                            )
                            # 1/Z = exp(-ln Z) on the scalar engine (cheap:
                            # cost is free-size-bound, all 128 rows in one op)
                            lnzb = attnp.tile([P, T], F32, tag="lnzb")
                            nc.scalar.activation(lnzb[:], psZb[:], AF.Ln)
                            yield
                            recb = attnp.tile([P, T], BF16, tag="recb")
                            nc.scalar.activation(
                                recb[:], lnzb[:], AF.Exp, scale=-1.0)
                            nc.vector.tensor_tensor(
                                attnT[:, j, i, :], psaP[:], recb[:], ALU.mult)
                            yield

                    saT = fmp.tile([P, CT, IP, T], BF16, tag="fmA2")
                    for m in range(CT):
                        psj = ps5.tile([P, IP, T], F32, tag="ps5")
                        for k in range(CT):
                            nc.tensor.matmul(
                                psj[:], wp_r[:, k, ts(m)], attnT[:, k, :, :],
                                start=(k == 0), stop=(k == CT - 1),
                            )
                        nc.scalar.activation(
                            saT[:, m, :, :], psj[:], AF.Identity,
                            bias=bpc[:, m : m + 1],
                        )
                        yield
                    x1 = actp.tile([P, IP, TT, C], F32, tag="x1")
                    for i in range(IP):
                        for tt in range(TT):
                            psr = ps5.tile([P, C], BF16, tag="ps5t", bufs=2)
                            for ct in range(CT):
                                nc.tensor.transpose(
                                    psr[:, ts(ct)], saT[:, ct, i, ts(tt)], id_b[:]
                                )
                            nc.vector.tensor_tensor(
                                x1[:, i, tt, :], psr[:], x_t[:, i, tt, :], ALU.add
                            )
                            yield
                    xn2 = yield from ln_stats_gen(x1[:])
                    return x1, xn2

                def back(g, x1, xn2):
                    h2_ct = yield from ln_fm(xn2, g2c, be2c, tag="h2ct")
                    z = zp.tile([P, FT, IP, T], BF16, tag="z")
                    for m in range(FT):
                        psz = ps5.tile([P, IP, T], F32, tag="ps5")
                        for k in range(CT):
                            nc.tensor.matmul(
                                psz[:], w1_r[:, k, ts(m)], h2_ct[:, k, :, :],
                                start=(k == 0), stop=(k == CT - 1),
                            )
                        if m % 2 == 0:
                            nc.scalar.activation(
                                z[:, m, :, :], psz[:], AF.Relu,
                                bias=b1c[:, m : m + 1],
                            )
                        else:
                            nc.vector.tensor_scalar(
                                z[:, m, :, :], psz[:],
                                b1c[:, m : m + 1], 0.0,
                                ALU.add, ALU.max,
                            )
                        yield
                    yT = fmp.tile([P, CT, IP, T], BF16, tag="fmB2")
                    for m in range(CT):
                        psy = ps5.tile([P, IP, T], F32, tag="ps5")
                        for k in range(FT):
                            nc.tensor.matmul(
                                psy[:], w2_r[:, k, ts(m)], z[:, k, :, :],
                                start=(k == 0), stop=(k == FT - 1),
                            )
                        nc.scalar.activation(
                            yT[:, m, :, :], psy[:], AF.Identity,
                            bias=b2c[:, m : m + 1],
                        )
                        yield
                    for i in range(IP):
                        for tt in range(TT):
                            pso = ps5.tile([P, C], BF16, tag="ps5t", bufs=2)
                            for ct in range(CT):
                                nc.tensor.transpose(
                                    pso[:, ts(ct)], yT[:, ct, i, ts(tt)], id_b[:]
                                )
                            o = actp.tile([P, C], F32, tag="o")
                            nc.vector.tensor_tensor(
                                o[:], pso[:], x1[:, i, tt, :], ALU.add
                            )
                            nc.sync.dma_start(out_t[g * IP + i, ts(tt), :], o[:])
                            yield

                def run_all(gens, strides=None):
                    """Round-robin the generators (gen k advances on rounds
                    divisible by strides[k]); return list of returns."""
                    if strides is None:
                        strides = [1] * len(gens)
                    rets = {}
                    live = {id(gn): gn for gn in gens}
                    order = [(id(gn), st) for gn, st in zip(gens, strides)]
                    rnd = 0
                    while live:
                        for key, st in order:
                            gn = live.get(key)
                            if gn is None or rnd % st != 0:
                                continue
                            try:
                                next(gn)
                            except StopIteration as e:
                                rets[key] = e.value
                                del live[key]
                        rnd += 1
                    return [rets[id(gn)] for gn in gens]

                def stats_hct_gen(src_ap):
                    xn = yield from ln_stats_gen(src_ap)
                    h_ct = yield from ln_fm(xn, g1c, be1c)
                    return h_ct

                xs = {0: load_x(0), 1: load_x(1)}
                (hct_cur,) = run_all([stats_hct_gen(xs[0][:])])
                pending = None
                for g in range(NG):
                    if g + 2 < NG:
                        xs[g + 2] = load_x(g + 2)
                    gens = [front(g, xs[g], hct_cur)]
                    strides = [1]
                    if pending is not None:
                        gens.append(back(*pending))
                        strides.append(2)
                    if g + 1 < NG:
                        gens.append(stats_hct_gen(xs[g + 1][:]))
                        strides.append(4)
                    rets = run_all(gens, strides)
                    pending = (g,) + rets[0]
                    hct_cur = rets[-1] if g + 1 < NG else None
                    del xs[g]
                run_all([back(*pending)])
    return nc


_NC_CACHE = None


def _get_nc():
    global _NC_CACHE
    if _NC_CACHE is None:
        _NC_CACHE = build_nc()
    return _NC_CACHE


def _host_consts():
    ident = np.eye(P, dtype=ml_dtypes.bfloat16)
    # tri3 = [tri | ones | tri]: tri[s, t] = 1 where t >= s (valid causal
    # entries of a diagonal [s, t] block)
    s = np.arange(P)[:, None]
    t = np.arange(P)[None, :]
    tri = (t >= s).astype(np.float32)
    tri3 = np.concatenate([tri, np.ones((P, P), np.float32), tri], axis=1)
    return ident, tri3.astype(ml_dtypes.bfloat16)


def kernel(x, Wq, Wk, Wv, Wp, bp, W1, b1, W2, b2, g1, be1, g2, be2):
    bf = ml_dtypes.bfloat16
    x = np.ascontiguousarray(np.asarray(x, np.float32))
    WqF = np.ascontiguousarray(
        np.asarray(Wq, np.float32).transpose(1, 0, 2).reshape(C, C).astype(bf)
    )
    WkF = np.ascontiguousarray(
        np.asarray(Wk, np.float32).transpose(1, 0, 2).reshape(C, C).astype(bf)
    )
    WvF = np.ascontiguousarray(
        np.asarray(Wv, np.float32).transpose(1, 0, 2).reshape(C, C).astype(bf)
    )
    WpF = np.ascontiguousarray(np.asarray(Wp, np.float32).astype(bf))
    W1F = np.ascontiguousarray(np.asarray(W1, np.float32).astype(bf))
    W2F = np.ascontiguousarray(np.asarray(W2, np.float32).astype(bf))
    gb = np.ascontiguousarray(
        np.stack([
            np.asarray(g1, np.float32), np.asarray(be1, np.float32),
            np.asarray(g2, np.float32), np.asarray(be2, np.float32),
            np.asarray(bp, np.float32), np.asarray(b2, np.float32),
        ])
    )
    b1v = np.ascontiguousarray(np.asarray(b1, np.float32))
    ident, tri3 = _host_consts()

    nc = _get_nc()
    shared = {
        "wqf": WqF, "wkf": WkF, "wvf": WvF, "wpf": WpF,
        "w1f": W1F, "w2f": W2F, "gb": gb, "b1v": b1v,
        "ident": ident, "tri3": tri3,
    }
    in_maps = []
    for c in range(NCORES):
        m = dict(shared)
        m["x"] = np.ascontiguousarray(x[c * BL : (c + 1) * BL])
        in_maps.append(m)

    from concourse.bass_utils import run_bass_kernel_spmd

    res = run_bass_kernel_spmd(nc, in_maps, list(range(NCORES)))
    out = np.concatenate([res.results[c]["out"] for c in range(NCORES)], axis=0)
    return out.astype(np.float32)



# revision 21
# speedup vs baseline: 1.0246x; 1.0246x over previous
"""Transformer block (dense_transformer) on 8 TRN2 NeuronCores.

Strategy: pure data-parallel over batch (B=128 -> 16 items/core), weights
replicated. Per item, all linear layers run feature-major ([feat, T] with
feat on partitions); LayerNorm/softmax run token-major ([T, feat]).
Matmul datapath is bf16 (2x PE stream rate vs f32r); residual stream and
softmax statistics stay f32. The softmax normalize is fused into the
weight transpose on PE via a diagonal 1/rowsum matrix built on GpSimd.
LN stats for group g+1 are computed during group g so PE never idles at
group boundaries.
"""

import numpy as np
import ml_dtypes

import concourse.bass as bass
import concourse.mybir as mybir
from concourse.tile import TileContext
from concourse.vector_clock import ScopedClock

F32 = mybir.dt.float32
BF16 = mybir.dt.bfloat16
F8 = mybir.dt.float8e4
DR = mybir.MatmulPerfMode.DoubleRow
AF = mybir.ActivationFunctionType
AX = mybir.AxisListType
ALU = mybir.AluOpType

B, T, C, H, D = 128, 256, 384, 6, 64
F = 4 * C
NCORES = 8
BL = B // NCORES
P = 128
TT = T // P    # 2 token tiles
CT = C // P    # 3 channel tiles
FT = F // P    # 12 ffn-hidden tiles
H2 = H // 2    # head pairs
LN_EPS = 1e-5
CSCALE = float(C) ** -0.5
QKSCALE = CSCALE / 4096.0
NEG = -1.0e9


class PatchedTileContext(TileContext):
    """Workaround for this container's walrus: BIR instructions may carry at
    most ONE attached sem wait. Hoist extras into standalone waits."""

    def _hoist_multi_waits(self):
        nc = self.nc
        assert self.sems is not None
        sem_by_num = {s.num: s for s in self.sems.allocated().values()}
        for func in nc.m.functions:
            for blk in func.blocks:
                insts = blk.instructions
                i = 0
                while i < len(insts):
                    inst = insts[i]
                    si = inst.sync_info
                    waits = list(si.on_wait) if (si and si.on_wait) else []
                    if len(waits) <= 1:
                        i += 1
                        continue
                    hoist = waits[1:]
                    for w in hoist:
                        if not (
                            w.sync_type == "semaphore"
                            and w.wait_mode == "sem-ge-imm"
                            and w.id in sem_by_num
                        ):
                            raise RuntimeError(
                                f"cannot hoist waits on {inst.name}: {waits}"
                            )
                    del si.on_wait[1:]
                    engine = nc.engines[inst.engine]
                    new_insts = []
                    for w in hoist:
                        wi = engine.wait_ge(sem_by_num[w.id], w.wait_value)
                        new_insts.append(wi.ins)
                    cur_list = nc.cur_bb.bb.instructions
                    for ni in new_insts:
                        cur_list.remove(ni)
                    insts[i:i] = new_insts
                    i += len(new_insts) + 1

    def _drain_and_barrier(self, tick_clock, wait_clock):
        nc = self.nc
        self._hoist_multi_waits()

        drain_inst = nc.sync.drain()
        wait_clock.add_sem_waits(
            drain_inst.ins, ScopedClock({None: tick_clock.global_clock})
        )
        waits = list(drain_inst.ins.sync_info.on_wait or [])
        if len(waits) > 1:
            drain_inst.ins.sync_info.on_wait.clear()
            assert self.sems is not None
            sem_by_num = {s.num: s for s in self.sems.allocated().values()}
            new_waits = []
            for w in waits:
                assert w.sync_type == "semaphore" and w.wait_mode == "sem-ge-imm", w
                new_waits.append(nc.sync.wait_ge(sem_by_num[w.id], w.wait_value))
            bb = nc.cur_bb.bb
            insts = bb.instructions
            names = [i.name for i in insts]
            di = names.index(drain_inst.ins.name)
            tail = insts[di + 1 : di + 1 + len(new_waits)]
            assert len(tail) == len(new_waits)
            insts[di : di + 1 + len(new_waits)] = tail + [drain_inst.ins]

        nc.all_engine_barrier()
        assert self.sems is not None
        popped = nc._tile_sem_poison_stack.pop()
        assert popped is self._sem_poison
        nc.clear_and_free_semaphores(list(self.sems.allocated().values()))
        nc.all_engine_barrier()


def ts(i, n=P):
    return slice(i * n, (i + 1) * n)


def build_nc():
    nc = bass.Bass()
    x_in = nc.dram_tensor("x", [BL, T, C], F32, kind="ExternalInput")
    wq_in = nc.dram_tensor("wqf", [C, C], F8, kind="ExternalInput")
    wk_in = nc.dram_tensor("wkf", [C, C], F8, kind="ExternalInput")
    wv_in = nc.dram_tensor("wvf", [C, C], F8, kind="ExternalInput")
    wp_in = nc.dram_tensor("wpf", [C, C], F8, kind="ExternalInput")
    w1_in = nc.dram_tensor("w1f", [C, F], F8, kind="ExternalInput")
    w2_in = nc.dram_tensor("w2f", [F, C], F8, kind="ExternalInput")
    gb_in = nc.dram_tensor("gb", [6, C], F32, kind="ExternalInput")
    b1_in = nc.dram_tensor("b1v", [F], F32, kind="ExternalInput")
    id_in = nc.dram_tensor("ident", [P, P], BF16, kind="ExternalInput")
    tri_in = nc.dram_tensor("tri3", [P, 3 * P], BF16, kind="ExternalInput")
    out_t = nc.dram_tensor("out", [BL, T, C], F32, kind="ExternalOutput")

    IP = 2               # items per group
    NG = BL // IP        # groups
    W = IP * T           # moving width for feature-major matmuls (512)

    with PatchedTileContext(nc) as tc:
        with tc.tile_pool(name="consts", bufs=1) as consts:
            def load_w(ap_dram, kt, m, tag):
                w = consts.tile([P, kt, m], F8, tag=tag)
                nc.sync.dma_start(w[:], ap_dram.rearrange("(kt p) m -> p kt m", p=P))
                return w

            wq_r = load_w(wq_in[:], CT, C, "wq")
            wk_r = load_w(wk_in[:], CT, C, "wk")
            wv_r = load_w(wv_in[:], CT, C, "wv")
            wp_r = load_w(wp_in[:], CT, C, "wp")
            w1_r = load_w(w1_in[:], CT, F, "w1")
            w2_r = load_w(w2_in[:], FT, C, "w2")

            id_b = consts.tile([P, P], BF16, tag="idb")
            nc.sync.dma_start(id_b[:], id_in[:])
            tri3 = consts.tile([P, 3, P], BF16, tag="tri3")
            nc.sync.dma_start(tri3[:], tri_in.rearrange("p (b f) -> p b f", b=3))
            ones_t = consts.tile([P, P], BF16, tag="ones")
            nc.gpsimd.memset(ones_t[:], 1.0)
            gb = consts.tile([P, 6, CT], F32, tag="gb")
            nc.sync.dma_start(gb[:], gb_in.rearrange("g (ct p) -> p g ct", p=P))
            b1c = consts.tile([P, FT], F32, tag="b1c")
            nc.sync.dma_start(b1c[:], b1_in.rearrange("(ft p) -> p ft", p=P))
            epsc = consts.tile([P, 1], F32, tag="eps")
            nc.gpsimd.memset(epsc[:], LN_EPS)

            g1c = gb[:, 0, :]
            be1c = gb[:, 1, :]
            g2c = gb[:, 2, :]
            be2c = gb[:, 3, :]
            bpc = gb[:, 4, :]
            b2c = gb[:, 5, :]

            with (
                tc.tile_pool(name="xld", bufs=3) as xldp,
                tc.tile_pool(name="act", bufs=2) as actp,
                tc.tile_pool(name="xn", bufs=3) as xnp,
                tc.tile_pool(name="fm", bufs=2) as fmp,
                tc.tile_pool(name="zp", bufs=1) as zp,
                tc.tile_pool(name="attn", bufs=4) as attnp,
                tc.tile_pool(name="stats", bufs=8) as stats,
                tc.tile_pool(name="ps5", bufs=2, space="PSUM") as ps5,
                tc.tile_pool(name="ps2", bufs=2, space="PSUM") as ps2,
            ):
                def load_x(g):
                    xt = xldp.tile([P, IP, TT, C], F32, tag="x")
                    nc.sync.dma_start(
                        xt[:],
                        x_in[g * IP : (g + 1) * IP].rearrange(
                            "i (tt p) c -> p i tt c", p=P
                        ),
                    )
                    return xt

                def ln_stats_gen(src):
                    """src [P, IP, TT, C] f32 -> xn [P, 4, C] bf16 normalized
                    (no affine -- g/be folded into the ln_fm copies)."""
                    s4 = src.rearrange("p i tt c -> p (i tt) c")
                    nseg = IP * TT
                    bns = stats.tile([P, nseg, 6], F32, tag="bns")
                    for seg in range(nseg):
                        nc.vector.bn_stats(bns[:, seg, :], s4[:, seg, :])
                        if seg % 2 == 1:
                            yield
                    mv = stats.tile([P, nseg, 2], F32, tag="mv")
                    for seg in range(nseg):
                        nc.vector.bn_aggr(mv[:, seg, :], bns[:, seg, :])
                    yield
                    lnv = stats.tile([P, nseg], F32, tag="lnv")
                    nc.scalar.activation(lnv[:], mv[:, :, 1], AF.Ln, bias=epsc[:])
                    rstd = stats.tile([P, nseg], F32, tag="rstd")
                    nc.scalar.activation(rstd[:], lnv[:], AF.Exp, scale=-0.5)
                    yield
                    xn = xnp.tile([P, nseg, C], BF16, tag="xn")
                    for seg in range(nseg):
                        nc.vector.tensor_scalar(
                            xn[:, seg, :], s4[:, seg, :],
                            mv[:, seg, 0:1],
                            rstd[:, seg : seg + 1],
                            ALU.subtract, ALU.mult,
                        )
                        if seg % 2 == 1:
                            yield
                    return xn

                def ln_fm(xn, gcol, becol, tag="hct"):
                    """xn [P, 4, C] bf16 -> h_ct [P, CT, IP, T] bf16 w/ affine."""
                    h_ct = fmp.tile([P, CT, IP, T], F8, tag=tag, name="h_ct")
                    for ct in range(CT):
                        ps = ps5.tile([P, IP, T], BF16, tag="ps5t", bufs=2)
                        for i in range(IP):
                            for tt in range(TT):
                                nc.tensor.transpose(
                                    ps[:, i, ts(tt)],
                                    xn[:, i * TT + tt, ts(ct)],
                                    id_b[:],
                                )
                        nc.scalar.activation(
                            h_ct[:, ct, :, :], ps[:], AF.Identity,
                            bias=becol[:, ct : ct + 1],
                            scale=gcol[:, ct : ct + 1],
                        )
                        yield
                    return h_ct

                def front(g, x_t, h_ct):
                    qT = fmp.tile([P, CT, IP, T], BF16, tag="fmA")
                    kT = fmp.tile([P, CT, IP, T], BF16, tag="fmB")
                    for m in range(CT):
                        psq = ps5.tile([P, IP, T], F32, tag="ps5")
                        psk = ps5.tile([P, IP, T], F32, tag="ps5")
                        nc.tensor.matmul(
                            psq[:], wq_r[:, 0:2, ts(m)], h_ct[:, 0:2, :, :],
                            start=True, stop=False, perf_mode=DR,
                        )
                        nc.tensor.matmul(
                            psk[:], wk_r[:, 0:2, ts(m)], h_ct[:, 0:2, :, :],
                            start=True, stop=False, perf_mode=DR,
                        )
                        nc.tensor.matmul(
                            psq[:], wq_r[:, 2, ts(m)], h_ct[:, 2, :, :],
                            start=False, stop=True,
                        )
                        nc.tensor.matmul(
                            psk[:], wk_r[:, 2, ts(m)], h_ct[:, 2, :, :],
                            start=False, stop=True,
                        )
                        nc.vector.tensor_copy(qT[:, m, :, :], psq[:])
                        nc.vector.tensor_copy(kT[:, m, :, :], psk[:])
                        yield
                    v_sb = fmp.tile([P, IP, TT, C], BF16, tag="fmC")
                    for i in range(IP):
                        for st in range(TT):
                            psv = ps2.tile([P, C], F32, tag="pss")
                            nc.tensor.matmul(
                                psv[:], h_ct[:, 0:2, i, ts(st)],
                                wv_r[:, 0:2, :],
                                start=True, stop=False, perf_mode=DR,
                            )
                            nc.tensor.matmul(
                                psv[:], h_ct[:, 2, i, ts(st)], wv_r[:, 2, :],
                                start=False, stop=True,
                            )
                            if st == 0:
                                nc.scalar.copy(v_sb[:, i, st, :], psv[:])
                            else:
                                nc.vector.tensor_copy(v_sb[:, i, st, :], psv[:])
                            yield

                    attnT = fmp.tile([P, CT, IP, T], F8, tag="fmC2")
                    for i in range(IP):
                        for j in range(H2):
                            # heads A=2j (q/k/v at partitions/cols 0:64 of
                            # chunk j), B=2j+1 (64:128). Scores computed
                            # directly in [s, t] layout (stationary = kT
                            # s-block); head pair runs concurrently via PE
                            # row tiling (K=64 at row 0 / row 64).
                            hA, hB = 2 * j, 2 * j + 1
                            cA = slice(hA * 64, hA * 64 + 64)
                            cB = slice(hB * 64, hB * 64 + 64)
                            # pss layout [s, 3, 128]:
                            #  [:,0:2,:] = s-block0 x t 0:256
                            #  [:,2,:]   = s-block1 x t 128:256
                            pssA = ps2.tile([P, 3, P], F32, tag="pss")
                            pssB = ps2.tile([P, 3, P], F32, tag="pss")
                            nc.tensor.matmul(
                                pssA[:, 0:2, :], kT[0:64, j, i, ts(0)],
                                qT[0:64, j, i, :], start=True, stop=True,
                            )
                            nc.tensor.matmul(
                                pssB[:, 0:2, :], kT[64:128, j, i, ts(0)],
                                qT[64:128, j, i, :], start=True, stop=True,
                            )
                            nc.tensor.matmul(
                                pssA[:, 2, :], kT[0:64, j, i, ts(1)],
                                qT[0:64, j, i, ts(1)], start=True, stop=True,
                            )
                            nc.tensor.matmul(
                                pssB[:, 2, :], kT[64:128, j, i, ts(1)],
                                qT[64:128, j, i, ts(1)], start=True, stop=True,
                            )
                            weA = attnp.tile([P, 3, P], BF16, tag="weA")
                            weB = attnp.tile([P, 3, P], BF16, tag="weB")
                            nc.scalar.activation(
                                weA[:], pssA[:], AF.Exp, scale=QKSCALE)
                            nc.scalar.activation(
                                weB[:], pssB[:], AF.Exp, scale=QKSCALE)
                            # causal mask: zero the two diagonal blocks
                            # (tri3 = [tri | ones | tri])
                            nc.vector.tensor_tensor(
                                weA[:], weA[:], tri3[:], ALU.mult)
                            nc.vector.tensor_tensor(
                                weB[:], weB[:], tri3[:], ALU.mult)
                            yield
                            # Z[t] = sum_s w[s,t], broadcast to the head's 64
                            # partitions by a ones-matrix stationary (A rows
                            # 0:64, B rows 64:128 -> col-tiled concurrent)
                            psZb = ps2.tile([P, T], F32, tag="psa")
                            psaP = ps2.tile([P, T], F32, tag="psa")
                            nc.tensor.matmul(
                                psZb[0:64, :], ones_t[:, 0:64],
                                weA[:, 0:2, :], start=True, stop=False,
                            )
                            nc.tensor.matmul(
                                psZb[64:128, :], ones_t[:, 0:64],
                                weB[:, 0:2, :], start=True, stop=False,
                            )
                            nc.tensor.matmul(
                                psZb[0:64, P:], ones_t[:, 0:64],
                                weA[:, 2, :], start=False, stop=True,
                            )
                            nc.tensor.matmul(
                                psZb[64:128, P:], ones_t[:, 0:64],
                                weB[:, 2, :], start=False, stop=True,
                            )
                            # attn (unnormalized): out[d, t] = sum_s v w
                            nc.tensor.matmul(
                                psaP[0:64, :], v_sb[:, i, 0, cA],
                                weA[:, 0:2, :], start=True, stop=False,
                            )
                            nc.tensor.matmul(
                                psaP[64:128, :], v_sb[:, i, 0, cB],
                                weB[:, 0:2, :], start=True, stop=False,
                            )
                            nc.tensor.matmul(
                                psaP[0:64, P:], v_sb[:, i, 1, cA],
                                weA[:, 2, :], start=False, stop=True,
                            )
                            nc.tensor.matmul(
                                psaP[64:128, P:], v_sb[:, i, 1, cB],
                                weB[:, 2, :], start=False, stop=True,
                            )
                            # 1/Z = exp(-ln Z) on the scalar engine (cheap:
                            # cost is free-size-bound, all 128 rows in one op)
                            lnzb = attnp.tile([P, T], F32, tag="lnzb")
                            nc.scalar.activation(lnzb[:], psZb[:], AF.Ln)
                            yield
                            recb = attnp.tile([P, T], BF16, tag="recb")
                            nc.scalar.activation(
                                recb[:], lnzb[:], AF.Exp, scale=-1.0)
                            nc.vector.tensor_tensor(
                                attnT[:, j, i, :], psaP[:], recb[:], ALU.mult)
                            yield

                    saT = fmp.tile([P, CT, IP, T], BF16, tag="fmA2")
                    for m in range(CT):
                        psj = ps5.tile([P, IP, T], F32, tag="ps5")
                        nc.tensor.matmul(
                            psj[:], wp_r[:, 0:2, ts(m)], attnT[:, 0:2, :, :],
                            start=True, stop=False, perf_mode=DR,
                        )
                        nc.tensor.matmul(
                            psj[:], wp_r[:, 2, ts(m)], attnT[:, 2, :, :],
                            start=False, stop=True,
                        )
                        nc.scalar.activation(
                            saT[:, m, :, :], psj[:], AF.Identity,
                            bias=bpc[:, m : m + 1], scale=1.0 / 4096.0,
                        )
                        yield
                    x1 = actp.tile([P, IP, TT, C], F32, tag="x1")
                    for i in range(IP):
                        for tt in range(TT):
                            psr = ps5.tile([P, C], BF16, tag="ps5t", bufs=2)
                            for ct in range(CT):
                                nc.tensor.transpose(
                                    psr[:, ts(ct)], saT[:, ct, i, ts(tt)], id_b[:]
                                )
                            nc.vector.tensor_tensor(
                                x1[:, i, tt, :], psr[:], x_t[:, i, tt, :], ALU.add
                            )
                            yield
                    xn2 = yield from ln_stats_gen(x1[:])
                    return x1, xn2

                def back(g, x1, xn2):
                    h2_ct = yield from ln_fm(xn2, g2c, be2c, tag="h2ct")
                    z = zp.tile([P, FT, IP, T], F8, tag="z")
                    for m in range(FT):
                        psz = ps5.tile([P, IP, T], F32, tag="ps5")
                        nc.tensor.matmul(
                            psz[:], w1_r[:, 0:2, ts(m)], h2_ct[:, 0:2, :, :],
                            start=True, stop=False, perf_mode=DR,
                        )
                        nc.tensor.matmul(
                            psz[:], w1_r[:, 2, ts(m)], h2_ct[:, 2, :, :],
                            start=False, stop=True,
                        )
                        if m % 2 == 0:
                            nc.scalar.activation(
                                z[:, m, :, :], psz[:], AF.Relu,
                                bias=b1c[:, m : m + 1],
                            )
                        else:
                            nc.vector.tensor_scalar(
                                z[:, m, :, :], psz[:],
                                b1c[:, m : m + 1], 0.0,
                                ALU.add, ALU.max,
                            )
                        yield
                    yT = fmp.tile([P, CT, IP, T], BF16, tag="fmB2")
                    for m in range(CT):
                        psy = ps5.tile([P, IP, T], F32, tag="ps5")
                        for u in range(FT // 2):
                            nc.tensor.matmul(
                                psy[:], w2_r[:, 2 * u : 2 * u + 2, ts(m)],
                                z[:, 2 * u : 2 * u + 2, :, :],
                                start=(u == 0), stop=(u == FT // 2 - 1),
                                perf_mode=DR,
                            )
                        nc.scalar.activation(
                            yT[:, m, :, :], psy[:], AF.Identity,
                            bias=b2c[:, m : m + 1], scale=1.0 / 4096.0,
                        )
                        yield
                    for i in range(IP):
                        for tt in range(TT):
                            pso = ps5.tile([P, C], BF16, tag="ps5t", bufs=2)
                            for ct in range(CT):
                                nc.tensor.transpose(
                                    pso[:, ts(ct)], yT[:, ct, i, ts(tt)], id_b[:]
                                )
                            o = actp.tile([P, C], F32, tag="o")
                            nc.vector.tensor_tensor(
                                o[:], pso[:], x1[:, i, tt, :], ALU.add
                            )
                            nc.sync.dma_start(out_t[g * IP + i, ts(tt), :], o[:])
                            yield

                def run_all(gens, strides=None):
                    """Round-robin the generators (gen k advances on rounds
                    divisible by strides[k]); return list of returns."""
                    if strides is None:
                        strides = [1] * len(gens)
                    rets = {}
                    live = {id(gn): gn for gn in gens}
                    order = [(id(gn), st) for gn, st in zip(gens, strides)]
                    rnd = 0
                    while live:
                        for key, st in order:
                            gn = live.get(key)
                            if gn is None or rnd % st != 0:
                                continue
                            try:
                                next(gn)
                            except StopIteration as e:
                                rets[key] = e.value
                                del live[key]
                        rnd += 1
                    return [rets[id(gn)] for gn in gens]

                def stats_hct_gen(src_ap):
                    xn = yield from ln_stats_gen(src_ap)
                    h_ct = yield from ln_fm(xn, g1c, be1c)
                    return h_ct

                xs = {0: load_x(0), 1: load_x(1)}
                (hct_cur,) = run_all([stats_hct_gen(xs[0][:])])
                pending = None
                for g in range(NG):
                    if g + 2 < NG:
                        xs[g + 2] = load_x(g + 2)
                    gens = [front(g, xs[g], hct_cur)]
                    strides = [1]
                    if pending is not None:
                        gens.append(back(*pending))
                        strides.append(2)
                    if g + 1 < NG:
                        gens.append(stats_hct_gen(xs[g + 1][:]))
                        strides.append(4)
                    rets = run_all(gens, strides)
                    pending = (g,) + rets[0]
                    hct_cur = rets[-1] if g + 1 < NG else None
                    del xs[g]
                run_all([back(*pending)])
    return nc


_NC_CACHE = None


def _get_nc():
    global _NC_CACHE
    if _NC_CACHE is None:
        _NC_CACHE = build_nc()
    return _NC_CACHE


def _host_consts():
    ident = np.eye(P, dtype=ml_dtypes.bfloat16)
    # tri3 = [tri | ones | tri]: tri[s, t] = 1 where t >= s (valid causal
    # entries of a diagonal [s, t] block)
    s = np.arange(P)[:, None]
    t = np.arange(P)[None, :]
    tri = (t >= s).astype(np.float32)
    tri3 = np.concatenate([tri, np.ones((P, P), np.float32), tri], axis=1)
    return ident, tri3.astype(ml_dtypes.bfloat16)


def kernel(x, Wq, Wk, Wv, Wp, bp, W1, b1, W2, b2, g1, be1, g2, be2):
    # Weights are prescaled by WS and stored fp8 e4m3; the matching 1/WS
    # (or 1/WS^2) rescale rides the psum-evacuating activations.
    f8 = ml_dtypes.float8_e4m3
    WS = 64.0
    x = np.ascontiguousarray(np.asarray(x, np.float32))

    def to8(w):
        w = np.asarray(w, np.float32) * WS
        assert np.abs(w).max() < 200, np.abs(w).max()
        return np.ascontiguousarray(w.astype(f8))

    WqF = to8(np.asarray(Wq, np.float32).transpose(1, 0, 2).reshape(C, C))
    WkF = to8(np.asarray(Wk, np.float32).transpose(1, 0, 2).reshape(C, C))
    WvF = to8(np.asarray(Wv, np.float32).transpose(1, 0, 2).reshape(C, C))
    WpF = to8(Wp)
    W1F = to8(W1)
    W2F = to8(W2)
    gb = np.ascontiguousarray(
        np.stack([
            np.asarray(g1, np.float32), np.asarray(be1, np.float32),
            np.asarray(g2, np.float32), np.asarray(be2, np.float32),
            np.asarray(bp, np.float32), np.asarray(b2, np.float32),
        ])
    )
    b1v = np.ascontiguousarray(np.asarray(b1, np.float32) * WS)
    ident, tri3 = _host_consts()

    nc = _get_nc()
    shared = {
        "wqf": WqF, "wkf": WkF, "wvf": WvF, "wpf": WpF,
        "w1f": W1F, "w2f": W2F, "gb": gb, "b1v": b1v,
        "ident": ident, "tri3": tri3,
    }
    in_maps = []
    for c in range(NCORES):
        m = dict(shared)
        m["x"] = np.ascontiguousarray(x[c * BL : (c + 1) * BL])
        in_maps.append(m)

    from concourse.bass_utils import run_bass_kernel_spmd

    res = run_bass_kernel_spmd(nc, in_maps, list(range(NCORES)))
    out = np.concatenate([res.results[c]["out"] for c in range(NCORES)], axis=0)
    return out.astype(np.float32)



# revision 23
# speedup vs baseline: 1.0541x; 1.0288x over previous
"""Transformer block (dense_transformer) on 8 TRN2 NeuronCores.

Strategy: pure data-parallel over batch (B=128 -> 16 items/core), weights
replicated. Per item, all linear layers run feature-major ([feat, T] with
feat on partitions); LayerNorm/softmax run token-major ([T, feat]).
Matmul datapath is bf16 (2x PE stream rate vs f32r); residual stream and
softmax statistics stay f32. The softmax normalize is fused into the
weight transpose on PE via a diagonal 1/rowsum matrix built on GpSimd.
LN stats for group g+1 are computed during group g so PE never idles at
group boundaries.
"""

import numpy as np
import ml_dtypes

import concourse.bass as bass
import concourse.mybir as mybir
from concourse.tile import TileContext
from concourse.vector_clock import ScopedClock

F32 = mybir.dt.float32
BF16 = mybir.dt.bfloat16
F8 = mybir.dt.float8e4
DR = mybir.MatmulPerfMode.DoubleRow
AF = mybir.ActivationFunctionType
AX = mybir.AxisListType
ALU = mybir.AluOpType

B, T, C, H, D = 128, 256, 384, 6, 64
F = 4 * C
NCORES = 8
BL = B // NCORES
P = 128
TT = T // P    # 2 token tiles
CT = C // P    # 3 channel tiles
FT = F // P    # 12 ffn-hidden tiles
H2 = H // 2    # head pairs
LN_EPS = 1e-5
CSCALE = float(C) ** -0.5
QKSCALE = CSCALE / 4096.0
NEG = -1.0e9


class PatchedTileContext(TileContext):
    """Workaround for this container's walrus: BIR instructions may carry at
    most ONE attached sem wait. Hoist extras into standalone waits."""

    def _hoist_multi_waits(self):
        nc = self.nc
        assert self.sems is not None
        sem_by_num = {s.num: s for s in self.sems.allocated().values()}
        for func in nc.m.functions:
            for blk in func.blocks:
                insts = blk.instructions
                i = 0
                while i < len(insts):
                    inst = insts[i]
                    si = inst.sync_info
                    waits = list(si.on_wait) if (si and si.on_wait) else []
                    if len(waits) <= 1:
                        i += 1
                        continue
                    hoist = waits[1:]
                    for w in hoist:
                        if not (
                            w.sync_type == "semaphore"
                            and w.wait_mode == "sem-ge-imm"
                            and w.id in sem_by_num
                        ):
                            raise RuntimeError(
                                f"cannot hoist waits on {inst.name}: {waits}"
                            )
                    del si.on_wait[1:]
                    engine = nc.engines[inst.engine]
                    new_insts = []
                    for w in hoist:
                        wi = engine.wait_ge(sem_by_num[w.id], w.wait_value)
                        new_insts.append(wi.ins)
                    cur_list = nc.cur_bb.bb.instructions
                    for ni in new_insts:
                        cur_list.remove(ni)
                    insts[i:i] = new_insts
                    i += len(new_insts) + 1

    def _drain_and_barrier(self, tick_clock, wait_clock):
        nc = self.nc
        self._hoist_multi_waits()

        drain_inst = nc.sync.drain()
        wait_clock.add_sem_waits(
            drain_inst.ins, ScopedClock({None: tick_clock.global_clock})
        )
        waits = list(drain_inst.ins.sync_info.on_wait or [])
        if len(waits) > 1:
            drain_inst.ins.sync_info.on_wait.clear()
            assert self.sems is not None
            sem_by_num = {s.num: s for s in self.sems.allocated().values()}
            new_waits = []
            for w in waits:
                assert w.sync_type == "semaphore" and w.wait_mode == "sem-ge-imm", w
                new_waits.append(nc.sync.wait_ge(sem_by_num[w.id], w.wait_value))
            bb = nc.cur_bb.bb
            insts = bb.instructions
            names = [i.name for i in insts]
            di = names.index(drain_inst.ins.name)
            tail = insts[di + 1 : di + 1 + len(new_waits)]
            assert len(tail) == len(new_waits)
            insts[di : di + 1 + len(new_waits)] = tail + [drain_inst.ins]

        nc.all_engine_barrier()
        assert self.sems is not None
        popped = nc._tile_sem_poison_stack.pop()
        assert popped is self._sem_poison
        nc.clear_and_free_semaphores(list(self.sems.allocated().values()))
        nc.all_engine_barrier()


def ts(i, n=P):
    return slice(i * n, (i + 1) * n)


def build_nc():
    nc = bass.Bass()
    x_in = nc.dram_tensor("x", [BL, T, C], F32, kind="ExternalInput")
    wq_in = nc.dram_tensor("wqf", [C, C], F8, kind="ExternalInput")
    wk_in = nc.dram_tensor("wkf", [C, C], F8, kind="ExternalInput")
    wv_in = nc.dram_tensor("wvf", [C, C], F8, kind="ExternalInput")
    wp_in = nc.dram_tensor("wpf", [C, C], F8, kind="ExternalInput")
    w1_in = nc.dram_tensor("w1f", [C, F], F8, kind="ExternalInput")
    w2_in = nc.dram_tensor("w2f", [F, C], F8, kind="ExternalInput")
    gb_in = nc.dram_tensor("gb", [6, C], F32, kind="ExternalInput")
    b1_in = nc.dram_tensor("b1v", [F], F32, kind="ExternalInput")
    id_in = nc.dram_tensor("ident", [P, P], BF16, kind="ExternalInput")
    mn_in = nc.dram_tensor("mneg", [P, P], BF16, kind="ExternalInput")
    out_t = nc.dram_tensor("out", [BL, T, C], F32, kind="ExternalOutput")

    IP = 2               # items per group
    NG = BL // IP        # groups
    W = IP * T           # moving width for feature-major matmuls (512)

    with PatchedTileContext(nc) as tc:
        with tc.tile_pool(name="consts", bufs=1) as consts:
            wq_r = consts.tile([P, CT, C], F8, tag="wq")
            wk_r = consts.tile([P, CT, C], F8, tag="wk")
            wv_r = consts.tile([P, CT, C], F8, tag="wv")
            wp_r = consts.tile([P, CT, C], F8, tag="wp")
            w1_r = consts.tile([P, CT, F], F8, tag="w1")
            w2_r = consts.tile([P, FT, C], F8, tag="w2")

            def dma_w(w, ap_dram):
                nc.sync.dma_start(
                    w[:], ap_dram.rearrange("(kt p) m -> p kt m", p=P))

            # small consts first so the LN/stats path unblocks immediately
            id_b = consts.tile([P, P], BF16, tag="idb")
            nc.sync.dma_start(id_b[:], id_in[:])
            gb = consts.tile([P, 6, CT], F32, tag="gb")
            nc.sync.dma_start(gb[:], gb_in.rearrange("g (ct p) -> p g ct", p=P))
            epsc = consts.tile([P, 1], F32, tag="eps")
            nc.gpsimd.memset(epsc[:], LN_EPS)
            mneg = consts.tile([P, P], BF16, tag="mneg")
            nc.sync.dma_start(mneg[:], mn_in[:])
            ones_t = consts.tile([P, P], BF16, tag="ones")
            nc.gpsimd.memset(ones_t[:], 1.0)
            b1c = consts.tile([P, FT], F32, tag="b1c")
            nc.sync.dma_start(b1c[:], b1_in.rearrange("(ft p) -> p ft", p=P))

            g1c = gb[:, 0, :]
            be1c = gb[:, 1, :]
            g2c = gb[:, 2, :]
            be2c = gb[:, 3, :]
            bpc = gb[:, 4, :]
            b2c = gb[:, 5, :]

            with (
                tc.tile_pool(name="xld", bufs=3) as xldp,
                tc.tile_pool(name="act", bufs=2) as actp,
                tc.tile_pool(name="xn", bufs=3) as xnp,
                tc.tile_pool(name="fm", bufs=2) as fmp,
                tc.tile_pool(name="zp", bufs=1) as zp,
                tc.tile_pool(name="attn", bufs=4) as attnp,
                tc.tile_pool(name="stats", bufs=8) as stats,
                tc.tile_pool(name="ps5", bufs=2, space="PSUM") as ps5,
                tc.tile_pool(name="ps2", bufs=2, space="PSUM") as ps2,
            ):
                def load_x(g):
                    xt = xldp.tile([P, IP, TT, C], F32, tag="x")
                    for i in range(IP):
                        nc.sync.dma_start(
                            xt[:, i],
                            x_in[g * IP + i].rearrange(
                                "(tt p) c -> p tt c", p=P),
                        )
                    return xt

                def ln_stats_gen(src):
                    """src [P, IP, TT, C] f32 -> xn [P, 4, C] bf16 normalized
                    (no affine -- g/be folded into the ln_fm copies)."""
                    s4 = src.rearrange("p i tt c -> p (i tt) c")
                    nseg = IP * TT
                    bns = stats.tile([P, nseg, 6], F32, tag="bns")
                    for seg in range(nseg):
                        nc.vector.bn_stats(bns[:, seg, :], s4[:, seg, :])
                        if seg % 2 == 1:
                            yield
                    mv = stats.tile([P, nseg, 2], F32, tag="mv")
                    for seg in range(nseg):
                        nc.vector.bn_aggr(mv[:, seg, :], bns[:, seg, :])
                    yield
                    lnv = stats.tile([P, nseg], F32, tag="lnv")
                    nc.scalar.activation(lnv[:], mv[:, :, 1], AF.Ln, bias=epsc[:])
                    rstd = stats.tile([P, nseg], F32, tag="rstd")
                    nc.scalar.activation(rstd[:], lnv[:], AF.Exp, scale=-0.5)
                    yield
                    xn = xnp.tile([P, nseg, C], BF16, tag="xn")
                    for seg in range(nseg):
                        nc.vector.tensor_scalar(
                            xn[:, seg, :], s4[:, seg, :],
                            mv[:, seg, 0:1],
                            rstd[:, seg : seg + 1],
                            ALU.subtract, ALU.mult,
                        )
                        if seg % 2 == 1:
                            yield
                    return xn

                def ln_fm(xn, gcol, becol, tag="hct"):
                    """xn [P, 4, C] bf16 -> h_ct [P, CT, IP, T] bf16 w/ affine."""
                    h_ct = fmp.tile([P, CT, IP, T], F8, tag=tag, name="h_ct")
                    for ct in range(CT):
                        ps = ps5.tile([P, IP, T], BF16, tag="ps5t", bufs=2)
                        for i in range(IP):
                            for tt in range(TT):
                                nc.tensor.transpose(
                                    ps[:, i, ts(tt)],
                                    xn[:, i * TT + tt, ts(ct)],
                                    id_b[:],
                                )
                        nc.scalar.activation(
                            h_ct[:, ct, :, :], ps[:], AF.Identity,
                            bias=becol[:, ct : ct + 1],
                            scale=gcol[:, ct : ct + 1],
                        )
                        yield
                    return h_ct

                def front(g, x_t, h_ct):
                    qT = fmp.tile([P, CT, IP, T], BF16, tag="fmA")
                    kT = fmp.tile([P, CT, IP, T], BF16, tag="fmB")
                    for m in range(CT):
                        psq = ps5.tile([P, IP, T], F32, tag="ps5")
                        psk = ps5.tile([P, IP, T], F32, tag="ps5")
                        nc.tensor.matmul(
                            psq[:], wq_r[:, 0:2, ts(m)], h_ct[:, 0:2, :, :],
                            start=True, stop=False, perf_mode=DR,
                        )
                        nc.tensor.matmul(
                            psk[:], wk_r[:, 0:2, ts(m)], h_ct[:, 0:2, :, :],
                            start=True, stop=False, perf_mode=DR,
                        )
                        nc.tensor.matmul(
                            psq[:], wq_r[:, 2, ts(m)], h_ct[:, 2, :, :],
                            start=False, stop=True,
                        )
                        nc.tensor.matmul(
                            psk[:], wk_r[:, 2, ts(m)], h_ct[:, 2, :, :],
                            start=False, stop=True,
                        )
                        nc.vector.tensor_copy(qT[:, m, :, :], psq[:])
                        nc.vector.tensor_copy(kT[:, m, :, :], psk[:])
                        yield
                    v_sb = fmp.tile([P, IP, TT, C], BF16, tag="fmC")
                    for i in range(IP):
                        for st in range(TT):
                            psv = ps2.tile([P, C], F32, tag="pss")
                            nc.tensor.matmul(
                                psv[:], h_ct[:, 0:2, i, ts(st)],
                                wv_r[:, 0:2, :],
                                start=True, stop=False, perf_mode=DR,
                            )
                            nc.tensor.matmul(
                                psv[:], h_ct[:, 2, i, ts(st)], wv_r[:, 2, :],
                                start=False, stop=True,
                            )
                            if st == 0:
                                nc.scalar.copy(v_sb[:, i, st, :], psv[:])
                            else:
                                nc.vector.tensor_copy(v_sb[:, i, st, :], psv[:])
                            yield

                    attnT = fmp.tile([P, CT, IP, T], F8, tag="fmC2")
                    for i in range(IP):
                        for j in range(H2):
                            # heads A=2j (q/k/v at partitions/cols 0:64 of
                            # chunk j), B=2j+1 (64:128). Scores computed
                            # directly in [s, t] layout (stationary = kT
                            # s-block); head pair runs concurrently via PE
                            # row tiling (K=64 at row 0 / row 64).
                            hA, hB = 2 * j, 2 * j + 1
                            cA = slice(hA * 64, hA * 64 + 64)
                            cB = slice(hB * 64, hB * 64 + 64)
                            # pss layout [s, 3, 128]:
                            #  [:,0:2,:] = s-block0 x t 0:256
                            #  [:,2,:]   = s-block1 x t 128:256
                            pssA = ps2.tile([P, 3, P], F32, tag="pss")
                            pssB = ps2.tile([P, 3, P], F32, tag="pss")
                            nc.tensor.matmul(
                                pssA[:, 0:2, :], kT[0:64, j, i, ts(0)],
                                qT[0:64, j, i, :], start=True, stop=False,
                            )
                            nc.tensor.matmul(
                                pssB[:, 0:2, :], kT[64:128, j, i, ts(0)],
                                qT[64:128, j, i, :], start=True, stop=False,
                            )
                            nc.tensor.matmul(
                                pssA[:, 2, :], kT[0:64, j, i, ts(1)],
                                qT[0:64, j, i, ts(1)], start=True, stop=False,
                            )
                            nc.tensor.matmul(
                                pssB[:, 2, :], kT[64:128, j, i, ts(1)],
                                qT[64:128, j, i, ts(1)], start=True, stop=False,
                            )
                            # causal mask: add NEG to the two diagonal
                            # blocks (id_b stationary, shared LDW)
                            nc.tensor.matmul(
                                pssA[:, 0, :], id_b[:], mneg[:],
                                start=False, stop=True,
                            )
                            nc.tensor.matmul(
                                pssB[:, 0, :], id_b[:], mneg[:],
                                start=False, stop=True,
                            )
                            nc.tensor.matmul(
                                pssA[:, 2, :], id_b[:], mneg[:],
                                start=False, stop=True,
                            )
                            nc.tensor.matmul(
                                pssB[:, 2, :], id_b[:], mneg[:],
                                start=False, stop=True,
                            )
                            weA = attnp.tile([P, 3, P], BF16, tag="weA")
                            weB = attnp.tile([P, 3, P], BF16, tag="weB")
                            nc.scalar.activation(
                                weA[:], pssA[:], AF.Exp, scale=QKSCALE)
                            nc.scalar.activation(
                                weB[:], pssB[:], AF.Exp, scale=QKSCALE)
                            yield
                            # Z[t] = sum_s w[s,t], broadcast to the head's 64
                            # partitions by a ones-matrix stationary (A rows
                            # 0:64, B rows 64:128 -> col-tiled concurrent)
                            psZb = ps2.tile([P, T], F32, tag="psa")
                            psaP = ps2.tile([P, T], F32, tag="psa")
                            nc.tensor.matmul(
                                psZb[0:64, :], ones_t[:, 0:64],
                                weA[:, 0:2, :], start=True, stop=False,
                            )
                            nc.tensor.matmul(
                                psZb[64:128, :], ones_t[:, 0:64],
                                weB[:, 0:2, :], start=True, stop=False,
                            )
                            nc.tensor.matmul(
                                psZb[0:64, P:], ones_t[:, 0:64],
                                weA[:, 2, :], start=False, stop=True,
                            )
                            nc.tensor.matmul(
                                psZb[64:128, P:], ones_t[:, 0:64],
                                weB[:, 2, :], start=False, stop=True,
                            )
                            # attn (unnormalized): out[d, t] = sum_s v w
                            nc.tensor.matmul(
                                psaP[0:64, :], v_sb[:, i, 0, cA],
                                weA[:, 0:2, :], start=True, stop=False,
                            )
                            nc.tensor.matmul(
                                psaP[64:128, :], v_sb[:, i, 0, cB],
                                weB[:, 0:2, :], start=True, stop=False,
                            )
                            nc.tensor.matmul(
                                psaP[0:64, P:], v_sb[:, i, 1, cA],
                                weA[:, 2, :], start=False, stop=True,
                            )
                            nc.tensor.matmul(
                                psaP[64:128, P:], v_sb[:, i, 1, cB],
                                weB[:, 2, :], start=False, stop=True,
                            )
                            # 1/Z = exp(-ln Z) on the scalar engine (cheap:
                            # cost is free-size-bound, all 128 rows in one op)
                            lnzb = attnp.tile([P, T], F32, tag="lnzb")
                            nc.scalar.activation(lnzb[:], psZb[:], AF.Ln)
                            yield
                            recb = attnp.tile([P, T], BF16, tag="recb")
                            nc.scalar.activation(
                                recb[:], lnzb[:], AF.Exp, scale=-1.0)
                            nc.vector.tensor_tensor(
                                attnT[:, j, i, :], psaP[:], recb[:], ALU.mult)
                            yield

                    saT = fmp.tile([P, CT, IP, T], BF16, tag="fmA2")
                    for m in range(CT):
                        psj = ps5.tile([P, IP, T], F32, tag="ps5")
                        nc.tensor.matmul(
                            psj[:], wp_r[:, 0:2, ts(m)], attnT[:, 0:2, :, :],
                            start=True, stop=False, perf_mode=DR,
                        )
                        nc.tensor.matmul(
                            psj[:], wp_r[:, 2, ts(m)], attnT[:, 2, :, :],
                            start=False, stop=True,
                        )
                        nc.scalar.activation(
                            saT[:, m, :, :], psj[:], AF.Identity,
                            bias=bpc[:, m : m + 1], scale=1.0 / 4096.0,
                        )
                        yield
                    x1 = actp.tile([P, IP, TT, C], F32, tag="x1")
                    for i in range(IP):
                        for tt in range(TT):
                            psr = ps5.tile([P, C], BF16, tag="ps5t", bufs=2)
                            for ct in range(CT):
                                nc.tensor.transpose(
                                    psr[:, ts(ct)], saT[:, ct, i, ts(tt)], id_b[:]
                                )
                            nc.vector.tensor_tensor(
                                x1[:, i, tt, :], psr[:], x_t[:, i, tt, :], ALU.add
                            )
                            yield
                    xn2 = yield from ln_stats_gen(x1[:])
                    return x1, xn2

                def back(g, x1, xn2):
                    h2_ct = yield from ln_fm(xn2, g2c, be2c, tag="h2ct")
                    z = zp.tile([P, FT, IP, T], F8, tag="z")
                    for m in range(FT):
                        psz = ps5.tile([P, IP, T], F32, tag="ps5")
                        nc.tensor.matmul(
                            psz[:], w1_r[:, 0:2, ts(m)], h2_ct[:, 0:2, :, :],
                            start=True, stop=False, perf_mode=DR,
                        )
                        nc.tensor.matmul(
                            psz[:], w1_r[:, 2, ts(m)], h2_ct[:, 2, :, :],
                            start=False, stop=True,
                        )
                        if m % 2 == 0:
                            nc.scalar.activation(
                                z[:, m, :, :], psz[:], AF.Relu,
                                bias=b1c[:, m : m + 1],
                            )
                        else:
                            nc.vector.tensor_scalar(
                                z[:, m, :, :], psz[:],
                                b1c[:, m : m + 1], 0.0,
                                ALU.add, ALU.max,
                            )
                        yield
                    yT = fmp.tile([P, CT, IP, T], BF16, tag="fmB2")
                    for m in range(CT):
                        psy = ps5.tile([P, IP, T], F32, tag="ps5")
                        for u in range(FT // 2):
                            nc.tensor.matmul(
                                psy[:], w2_r[:, 2 * u : 2 * u + 2, ts(m)],
                                z[:, 2 * u : 2 * u + 2, :, :],
                                start=(u == 0), stop=(u == FT // 2 - 1),
                                perf_mode=DR,
                            )
                        nc.scalar.activation(
                            yT[:, m, :, :], psy[:], AF.Identity,
                            bias=b2c[:, m : m + 1], scale=1.0 / 4096.0,
                        )
                        yield
                    for i in range(IP):
                        for tt in range(TT):
                            pso = ps5.tile([P, C], BF16, tag="ps5t", bufs=2)
                            for ct in range(CT):
                                nc.tensor.transpose(
                                    pso[:, ts(ct)], yT[:, ct, i, ts(tt)], id_b[:]
                                )
                            o = actp.tile([P, C], F32, tag="o")
                            nc.vector.tensor_tensor(
                                o[:], pso[:], x1[:, i, tt, :], ALU.add
                            )
                            nc.sync.dma_start(out_t[g * IP + i, ts(tt), :], o[:])
                            yield

                def run_all(gens, strides=None):
                    """Round-robin the generators (gen k advances on rounds
                    divisible by strides[k]); return list of returns."""
                    if strides is None:
                        strides = [1] * len(gens)
                    rets = {}
                    live = {id(gn): gn for gn in gens}
                    order = [(id(gn), st) for gn, st in zip(gens, strides)]
                    rnd = 0
                    while live:
                        for key, st in order:
                            gn = live.get(key)
                            if gn is None or rnd % st != 0:
                                continue
                            try:
                                next(gn)
                            except StopIteration as e:
                                rets[key] = e.value
                                del live[key]
                        rnd += 1
                    return [rets[id(gn)] for gn in gens]

                def stats_hct_gen(src_ap):
                    xn = yield from ln_stats_gen(src_ap)
                    h_ct = yield from ln_fm(xn, g1c, be1c)
                    return h_ct

                xs = {0: load_x(0)}
                dma_w(wq_r, wq_in[:])
                dma_w(wk_r, wk_in[:])
                xs[1] = load_x(1)
                dma_w(wv_r, wv_in[:])
                dma_w(wp_r, wp_in[:])
                dma_w(w1_r, w1_in[:])
                dma_w(w2_r, w2_in[:])
                (hct_cur,) = run_all([stats_hct_gen(xs[0][:])])
                pending = None
                for g in range(NG):
                    if g + 2 < NG:
                        xs[g + 2] = load_x(g + 2)
                    gens = [front(g, xs[g], hct_cur)]
                    strides = [1]
                    if pending is not None:
                        gens.append(back(*pending))
                        strides.append(2)
                    if g + 1 < NG:
                        gens.append(stats_hct_gen(xs[g + 1][:]))
                        strides.append(4)
                    rets = run_all(gens, strides)
                    pending = (g,) + rets[0]
                    hct_cur = rets[-1] if g + 1 < NG else None
                    del xs[g]
                run_all([back(*pending)])
    return nc


_NC_CACHE = None


def _get_nc():
    global _NC_CACHE
    if _NC_CACHE is None:
        _NC_CACHE = build_nc()
    return _NC_CACHE


def _host_consts():
    ident = np.eye(P, dtype=ml_dtypes.bfloat16)
    # mneg[s, t] = NEG where t < s (invalid causal entries of a diagonal
    # [s, t] block)
    s = np.arange(P)[:, None]
    t = np.arange(P)[None, :]
    mneg = np.where(t < s, NEG, 0.0).astype(np.float32)
    return ident, mneg.astype(ml_dtypes.bfloat16)


def kernel(x, Wq, Wk, Wv, Wp, bp, W1, b1, W2, b2, g1, be1, g2, be2):
    # Weights are prescaled by WS and stored fp8 e4m3; the matching 1/WS
    # (or 1/WS^2) rescale rides the psum-evacuating activations.
    f8 = ml_dtypes.float8_e4m3
    WS = 64.0
    x = np.ascontiguousarray(np.asarray(x, np.float32))

    def to8(w):
        w = np.asarray(w, np.float32) * WS
        assert np.abs(w).max() < 200, np.abs(w).max()
        return np.ascontiguousarray(w.astype(f8))

    WqF = to8(np.asarray(Wq, np.float32).transpose(1, 0, 2).reshape(C, C))
    WkF = to8(np.asarray(Wk, np.float32).transpose(1, 0, 2).reshape(C, C))
    WvF = to8(np.asarray(Wv, np.float32).transpose(1, 0, 2).reshape(C, C))
    WpF = to8(Wp)
    W1F = to8(W1)
    W2F = to8(W2)
    gb = np.ascontiguousarray(
        np.stack([
            np.asarray(g1, np.float32), np.asarray(be1, np.float32),
            np.asarray(g2, np.float32), np.asarray(be2, np.float32),
            np.asarray(bp, np.float32), np.asarray(b2, np.float32),
        ])
    )
    b1v = np.ascontiguousarray(np.asarray(b1, np.float32) * WS)
    ident, mneg = _host_consts()

    nc = _get_nc()
    shared = {
        "wqf": WqF, "wkf": WkF, "wvf": WvF, "wpf": WpF,
        "w1f": W1F, "w2f": W2F, "gb": gb, "b1v": b1v,
        "ident": ident, "mneg": mneg,
    }
    in_maps = []
    for c in range(NCORES):
        m = dict(shared)
        m["x"] = np.ascontiguousarray(x[c * BL : (c + 1) * BL])
        in_maps.append(m)

    from concourse.bass_utils import run_bass_kernel_spmd

    res = run_bass_kernel_spmd(nc, in_maps, list(range(NCORES)))
    out = np.concatenate([res.results[c]["out"] for c in range(NCORES)], axis=0)
    return out.astype(np.float32)

